# revision 1
# baseline (speedup 1.0000x reference)
"""Trainium2 Bass kernel for 16-head MHA with RoPE (dense_transformer).

Sharding: tensor-parallel over heads (2 heads/core on 8 cores) for
QKV projection + attention, then an AllToAll resharding to
token-parallel (512 tokens/core) for the output projection.

Layout strategy (per core, rank r):
  - x arrives column-sliced ([4096, 128] slice r); the core PE-transposes
    its slice and an AllGather materializes xT [1024, 4096] (dim-major
    activations) in DRAM, shared work across cores.
  - qT/kT/vT [128, 4096] are computed dim-major (feature dim on
    partitions) via  wT.T @ xT  matmuls in float32r.
  - RoPE is fused into the PSUM->SBUF evacuation: one tensor_tensor
    multiply by cos, one by a sign-folded sin table, plus 4 strip
    SBUF->SBUF accumulate-DMAs that implement rotate_half's partition
    rotation.
  - v is re-transposed to token-major [keys, 64] tiles with a ones
    column appended, so attention  out.T = [v | 1].T @ exp(S.T)  yields
    the softmax denominator as row 64 for free.
  - Scores are computed transposed (S.T = k.T^T @ qT per 128-key chunk),
    exp on ScalarE with the 1/sqrt(hd) scale folded in; no max
    subtraction (scores are bounded ~|9.3|, exp stays finite in fp32).
  - AllToAll reshards attention output from head-parallel to
    token-parallel; o-projection consumes the gathered aT dim-major.
"""

import numpy as np

# Problem shape (hardcoded per contract - kernel.py must be self-contained)
B, L_FULL, D = 2, 2048, 1024
H, HD = 16, 64
N_CORES = 8
HPC = H // N_CORES            # heads per core = 2
DPC = D // N_CORES            # xT dim-slice per core = 128
KC = D // 128                 # contraction chunks = 8


def _rope_tables(L):
    inv_freq = 1.0 / (10000.0 ** (np.arange(0, HD, 2, dtype=np.float64) / HD))
    t = np.arange(L, dtype=np.float64)
    freqs = np.outer(t, inv_freq)                      # [L, 32]
    emb = np.concatenate([freqs, freqs], -1)           # [L, 64]
    cos_t = np.cos(emb).T.astype(np.float32)           # [64, L]
    sin_t = np.sin(emb).T.astype(np.float32)
    cost = np.concatenate([cos_t, cos_t], 0)           # [128, L] (2 heads)
    sp = np.concatenate([sin_t[:32], -sin_t[32:]], 0)  # sign-folded
    sinp = np.concatenate([sp, sp], 0)                 # [128, L]
    return np.ascontiguousarray(cost), np.ascontiguousarray(sinp)


def build_mha(tc, L=L_FULL, debug=False):
    """Emit the MHA program into TileContext `tc`.

    Declares its own DRAM I/O tensors:
      in : xcol [B*L, 128], wqt/wkt/wvt [D, 128], wot [D, D]
      out: y [B*L/8, D]
    """
    import concourse.bass as bass
    import concourse.mybir as mybir
    from contextlib import ExitStack

    nc = tc.nc
    f32 = mybir.dt.float32
    f32r = mybir.dt.float32r
    AF = mybir.ActivationFunctionType
    ALU = mybir.AluOpType

    T = B * L                     # tokens
    TPC = T // N_CORES            # tokens per core (a2a shard width)
    CH = min(512, L)              # projection token-chunk (never crosses a batch)
    NCH = T // CH                 # projection chunks
    MC = L // 128                 # key chunks per batch
    FQ = min(1024, L)             # attention query tile (exp free-dim)
    NQ = min(512, FQ)             # matmul moving-dim tile
    NH = L // FQ                  # query tiles per batch
    MT = min(128, TPC)            # o-proj token tile
    scale = float(HD) ** -0.5
    rg = [list(range(N_CORES))]

    def r(ap):
        return ap.bitcast(f32r)

    # ---- I/O ----
    xcol_d = nc.dram_tensor("xcol", [T, DPC], f32, kind="ExternalInput").ap()
    wqt_d = nc.dram_tensor("wqt", [D, 128], f32, kind="ExternalInput").ap()
    wkt_d = nc.dram_tensor("wkt", [D, 128], f32, kind="ExternalInput").ap()
    wvt_d = nc.dram_tensor("wvt", [D, 128], f32, kind="ExternalInput").ap()
    wot_d = nc.dram_tensor("wot", [D, D], f32, kind="ExternalInput").ap()
    y_d = nc.dram_tensor("y", [TPC, D], f32, kind="ExternalOutput").ap()
    if debug:
        dbg_st0 = nc.dram_tensor("dbg_st0", [128, FQ], f32, kind="ExternalOutput").ap()
        dbg_st1 = nc.dram_tensor("dbg_st1", [128, FQ], f32, kind="ExternalOutput").ap()
        dbg_pt0 = nc.dram_tensor("dbg_pt0", [128, FQ], f32, kind="ExternalOutput").ap()
        dbg_pt1 = nc.dram_tensor("dbg_pt1", [128, FQ], f32, kind="ExternalOutput").ap()
        dbg_ou0 = nc.dram_tensor("dbg_ou0", [65, FQ], f32, kind="ExternalOutput").ap()
        dbg_dinv = nc.dram_tensor("dbg_dinv", [1, FQ], f32, kind="ExternalOutput").ap()
        dbg_ast = nc.dram_tensor("dbg_ast", [64, FQ], f32, kind="ExternalOutput").ap()
        dbg_qt = nc.dram_tensor("dbg_qt", [128, T], f32, kind="ExternalOutput").ap()
        dbg_kt = nc.dram_tensor("dbg_kt", [128, T], f32, kind="ExternalOutput").ap()
        dbg_vt = nc.dram_tensor("dbg_vt", [128, T], f32, kind="ExternalOutput").ap()
        dbg_ai = nc.dram_tensor("dbg_ai", [D, TPC], f32, kind="ExternalOutput").ap()
        dbg_ao = nc.dram_tensor("dbg_ao", [D, TPC], f32, kind="ExternalOutput").ap()
        dbg_ag = nc.dram_tensor("dbg_ag", [D, T], f32, kind="ExternalOutput").ap()

    # ---- inline constants ----
    cost_np, sinp_np = _rope_tables(L)
    ident_d = nc.inline_tensor(np.eye(128, dtype=np.float32), name="ident")
    cost_d = nc.inline_tensor(cost_np, name="cost")
    sinp_d = nc.inline_tensor(sinp_np, name="sinp")
    ones_d = nc.inline_tensor(np.ones((1, 64), dtype=np.float32), name="ones64")

    ctx = ExitStack()
    with ctx:
        # ---------------- persistent pools ----------------
        cpool = ctx.enter_context(tc.tile_pool(name="consts", bufs=1))
        ident = cpool.tile([128, 128], f32)
        nc.sync.dma_start(ident[:], ident_d.ap()[:, :])
        cost = cpool.tile([128, L], f32)
        nc.sync.dma_start(cost[:], cost_d.ap()[:, :])
        sinp = cpool.tile([128, L], f32)
        nc.sync.dma_start(sinp[:], sinp_d.ap()[:, :])
        ones64 = cpool.tile([1, 64], f32)
        nc.sync.dma_start(ones64[:], ones_d.ap()[:, :])

        dram = ctx.enter_context(tc.tile_pool(name="dram", bufs=1, space="DRAM"))
        ag_in = dram.tile([DPC, T], f32)
        ag_out = dram.tile([D, T], f32, addr_space="Shared")
        a2a_in = dram.tile([D, TPC], f32)
        a2a_out = dram.tile([D, TPC], f32)

        qkpool = ctx.enter_context(tc.tile_pool(name="qk", bufs=1))
        qt = qkpool.tile([128, T], f32)   # post-RoPE q, dim-major (f32 accum)
        kt = qkpool.tile([128, T], f32)
        qtr = qkpool.tile([128, T], f32r)  # rounded copies feeding matmuls
        ktr = qkpool.tile([128, T], f32r)

        # ---------------- stage 1: xT slice + AllGather ----------------
        with tc.tile_pool(name="xtr", bufs=1) as xtrp, \
             tc.tile_pool(name="xc", bufs=3) as xcp, \
             tc.tile_pool(name="tps", bufs=2, space="PSUM") as tpsp:
            xtr = xtrp.tile([DPC, T], f32)
            for c in range(T // 128):
                xc = xcp.tile([128, DPC], f32)
                nc.sync.dma_start(xc[:], xcol_d[c * 128:(c + 1) * 128, :])
                tps = tpsp.tile([DPC, 128], f32)
                nc.tensor.transpose(tps[:], xc[:], ident[:DPC, :128])
                nc.vector.tensor_copy(xtr[:, c * 128:(c + 1) * 128], tps[:])
            nc.sync.dma_start(ag_in[:, :], xtr[:])
        nc.gpsimd.collective_compute(
            "AllGather", ALU.bypass, ins=[ag_in.opt()], outs=[ag_out.opt()],
            replica_groups=rg,
        )

        # ---------------- stage 2+3: weights + projections ----------------
        vt_pool = ctx.enter_context(tc.tile_pool(name="vtp", bufs=1))
        vt = vt_pool.tile([128, T], f32)  # v dim-major (pre-transpose)

        with tc.tile_pool(name="wqkv", bufs=1) as wp, \
             tc.tile_pool(name="xt", bufs=2) as xtp, \
             tc.tile_pool(name="u", bufs=2) as up, \
             tc.tile_pool(name="pps", bufs=2, space="PSUM") as pps:
            wq_sb = wp.tile([128, KC, 128], f32r)
            wk_sb = wp.tile([128, KC, 128], f32r)
            wv_sb = wp.tile([128, KC, 128], f32r)
            for kk in range(KC):
                nc.sync.dma_start(wq_sb[:, kk, :],
                                  r(wqt_d[kk * 128:(kk + 1) * 128, :]))
                nc.sync.dma_start(wk_sb[:, kk, :],
                                  r(wkt_d[kk * 128:(kk + 1) * 128, :]))
                nc.sync.dma_start(wv_sb[:, kk, :],
                                  r(wvt_d[kk * 128:(kk + 1) * 128, :]))

            for c in range(NCH):
                l0 = (c * CH) % L   # position within batch (tables index)
                sl = slice(c * CH, (c + 1) * CH)
                xt = xtp.tile([128, KC, CH], f32r)
                for kk in range(KC):
                    nc.sync.dma_start(
                        xt[:, kk, :], r(ag_out[kk * 128:(kk + 1) * 128, sl]))
                q_ps = pps.tile([128, CH], f32, tag="q_ps")
                k_ps = pps.tile([128, CH], f32, tag="k_ps")
                v_ps = pps.tile([128, CH], f32, tag="v_ps")
                for kk in range(KC):
                    st_, sp_ = (kk == 0), (kk == KC - 1)
                    nc.tensor.matmul(q_ps[:], r(wq_sb[:, kk, :]), r(xt[:, kk, :]),
                                     start=st_, stop=sp_)
                    nc.tensor.matmul(k_ps[:], r(wk_sb[:, kk, :]), r(xt[:, kk, :]),
                                     start=st_, stop=sp_)
                    nc.tensor.matmul(v_ps[:], r(wv_sb[:, kk, :]), r(xt[:, kk, :]),
                                     start=st_, stop=sp_)
                # RoPE-fused evacuation for q and k
                tb = slice(l0, l0 + CH)
                for ps, dst, dstr in ((q_ps, qt, qtr), (k_ps, kt, ktr)):
                    u = up.tile([128, CH], f32, tag="u")
                    nc.vector.tensor_mul(u[:], ps[:], sinp[:, tb])
                    nc.vector.tensor_mul(dst[:, sl], ps[:], cost[:, tb])
                    for h in range(HPC):
                        a, b_ = h * 64, h * 64 + 32
                        c_ = h * 64 + 64
                        nc.gpsimd.dma_start(dst[a:b_, sl], u[b_:c_, :],
                                            accum_op=ALU.add)
                        nc.gpsimd.dma_start(dst[b_:c_, sl], u[a:b_, :],
                                            accum_op=ALU.add)
                    nc.vector.tensor_copy(dstr[:, sl], dst[:, sl])
                nc.vector.tensor_copy(vt[:, sl], v_ps[:])

        # ---------------- stage 4: v -> token-major [keys, 64|1] ----------
        vpool = ctx.enter_context(tc.tile_pool(name="vtm", bufs=1))
        v_sb = [vpool.tile([128, HPC, MC, 65], f32r, tag=f"v{b}", name=f"v_sb{b}")
                for b in range(B)]
        with tc.tile_pool(name="vps", bufs=2, space="PSUM") as vps, \
             tc.tile_pool(name="onc", bufs=1) as onc:
            ones_col = onc.tile([128, HPC, MC, 1], f32)
            nc.gpsimd.memset(ones_col[:], 1.0)
            for b in range(B):
                nc.vector.tensor_copy(v_sb[b][:, :, :, 64:65], ones_col[:])
                for h in range(HPC):
                    hs = slice(h * 64, (h + 1) * 64)
                    for m in range(MC):
                        ks = slice(b * L + m * 128, b * L + (m + 1) * 128)
                        vp = vps.tile([128, 64], f32)
                        nc.tensor.transpose(vp[:], vt[hs, ks], ident[hs, hs])
                        nc.vector.tensor_copy(v_sb[b][:, h, m, 0:64], vp[:])

        # ---------------- stage 5: attention ----------------
        s5 = ExitStack()
        epool = s5.enter_context(tc.tile_pool(name="ep", bufs=2))
        ptpool = s5.enter_context(tc.tile_pool(name="pt", bufs=2))
        stp = s5.enter_context(tc.tile_pool(name="stp", bufs=1, space="PSUM"))
        oup = s5.enter_context(tc.tile_pool(name="oup", bufs=1, space="PSUM"))

        for b in range(B):
            for nh in range(NH):
                q0 = b * L + nh * FQ          # global query offset
                outU = [oup.tile([65, FQ], f32, tag=f"outU{h}", name=f"outU{h}")
                        for h in range(HPC)]
                for m in range(MC):
                    ks = slice(b * L + m * 128, b * L + (m + 1) * 128)
                    sts = []
                    for h in range(HPC):
                        hs = slice(h * 64, (h + 1) * 64)
                        st = stp.tile([128, FQ], f32, tag=f"st{h}")
                        for n in range(FQ // NQ):
                            qs = slice(q0 + n * NQ, q0 + (n + 1) * NQ)
                            nc.tensor.matmul(st[:, n * NQ:(n + 1) * NQ],
                                             ktr[hs, ks], qtr[hs, qs],
                                             start=True, stop=True)
                        sts.append(st)
                    if debug and b == 0 and nh == 0 and m == 0:
                        for st_, d_ in ((sts[0], dbg_st0), (sts[1], dbg_st1)):
                            cp = epool.tile([128, FQ], f32, tag="dbgcp", name="cp")
                            nc.vector.tensor_copy(cp[:], st_[:])
                            nc.sync.dma_start(d_[:, :], cp[:])
                    pts = []
                    for h in range(HPC):
                        pt = ptpool.tile([128, FQ], f32r, tag=f"pt{h}")
                        nc.scalar.activation(pt[:], sts[h][:], AF.Exp, scale=scale)
                        pts.append(pt)
                    if debug and b == 0 and nh == 0 and m == 0:
                        nc.sync.dma_start(dbg_pt0[:, :], pts[0][:].bitcast(f32))
                        nc.sync.dma_start(dbg_pt1[:, :], pts[1][:].bitcast(f32))
                    for h in range(HPC):
                        for n in range(FQ // NQ):
                            ns = slice(n * NQ, (n + 1) * NQ)
                            nc.tensor.matmul(outU[h][:, ns],
                                             r(v_sb[b][:, h, m, :]), r(pts[h][:, ns]),
                                             start=(m == 0), stop=(m == MC - 1))
                # epilogue: normalize + stage into a2a input
                if debug and b == 0 and nh == 0:
                    cp2 = epool.tile([65, FQ], f32, tag="dbgcp2", name="cp2")
                    nc.vector.tensor_copy(cp2[:], outU[0][:])
                    nc.sync.dma_start(dbg_ou0[:, :], cp2[:])
                for h in range(HPC):
                    dinv = epool.tile([1, FQ], f32, tag="dinv")
                    nc.vector.reciprocal(dinv[:], outU[h][64:65, :])
                    if debug and b == 0 and nh == 0 and h == 0:
                        nc.sync.dma_start(dbg_dinv[:, :], dinv[:])
                    bc_sb = epool.tile([64, FQ], f32, tag="bc_sb")
                    nc.gpsimd.partition_broadcast(bc_sb[:], dinv[:])
                    a_st = epool.tile([64, FQ], f32, tag="a_st")
                    nc.vector.tensor_mul(a_st[:], outU[h][0:64, :], bc_sb[:])
                    if debug and b == 0 and nh == 0 and h == 0:
                        nc.sync.dma_start(dbg_ast[:, :], a_st[:])
                    for nq in range(FQ // TPC):
                        j = (b * L + nh * FQ) // TPC + nq
                        nc.sync.dma_start(
                            a2a_in[j * 128 + h * 64: j * 128 + (h + 1) * 64, :],
                            a_st[:, nq * TPC:(nq + 1) * TPC])

        s5.close()

        if debug:
            nc.sync.dma_start(dbg_qt[:, :], qt[:])
            nc.sync.dma_start(dbg_kt[:, :], kt[:])
            nc.sync.dma_start(dbg_vt[:, :], vt[:])
            nc.sync.dma_start(dbg_ai[:, :], a2a_in[:])
            nc.sync.dma_start(dbg_ag[:, :], ag_out[:])

        # ---------------- stage 6: AllToAll ----------------
        nc.gpsimd.collective_compute(
            "AllToAll", ALU.bypass, ins=[a2a_in.opt()], outs=[a2a_out.opt()],
            replica_groups=rg,
        )

        if debug:
            nc.sync.dma_start(dbg_ao[:, :], a2a_out[:])

        # ---------------- stage 7: o-projection ----------------
        with tc.tile_pool(name="wo", bufs=1) as wop, \
             tc.tile_pool(name="at", bufs=1) as atp, \
             tc.tile_pool(name="ysb", bufs=2) as ysp, \
             tc.tile_pool(name="yps", bufs=2, space="PSUM") as yps:
            wo_sb = wop.tile([128, KC, D], f32r)
            at_sb = atp.tile([128, KC, TPC], f32r)
            for kk in range(KC):
                nc.sync.dma_start(wo_sb[:, kk, :],
                                  r(wot_d[kk * 128:(kk + 1) * 128, :]))
                nc.sync.dma_start(at_sb[:, kk, :],
                                  r(a2a_out[kk * 128:(kk + 1) * 128, :]))
            for mt in range(TPC // MT):
                ms = slice(mt * MT, (mt + 1) * MT)
                y_sb = ysp.tile([MT, D], f32, tag="y_sb")
                for no in range(D // 512):
                    y_ps = yps.tile([MT, 512], f32, tag="y_ps")
                    for kk in range(KC):
                        nc.tensor.matmul(y_ps[:], r(at_sb[:, kk, ms]),
                                         r(wo_sb[:, kk, no * 512:(no + 1) * 512]),
                                         start=(kk == 0), stop=(kk == KC - 1))
                    nc.vector.tensor_copy(y_sb[:, no * 512:(no + 1) * 512], y_ps[:])
                nc.sync.dma_start(y_d[ms, :], y_sb[:])

    return nc


def make_in_maps(x, wq, wk, wv, wo, L=L_FULL):
    T = B * L
    x2 = np.ascontiguousarray(np.asarray(x, dtype=np.float32).reshape(T, D))
    wq = np.asarray(wq, dtype=np.float32)
    wk = np.asarray(wk, dtype=np.float32)
    wv = np.asarray(wv, dtype=np.float32)
    wo = np.asarray(wo, dtype=np.float32)
    wot = np.ascontiguousarray(wo.T)
    in_maps = []
    for rk in range(N_CORES):
        rows = slice(rk * 128, (rk + 1) * 128)
        in_maps.append({
            "xcol": np.ascontiguousarray(x2[:, rk * DPC:(rk + 1) * DPC]),
            "wqt": np.ascontiguousarray(wq[rows].T),
            "wkt": np.ascontiguousarray(wk[rows].T),
            "wvt": np.ascontiguousarray(wv[rows].T),
            "wot": wot,
        })
    return in_maps


_BUILT = {}


def _get_nc(L=L_FULL):
    if L not in _BUILT:
        import concourse.tile as tile
        from concourse import bacc
        nc = bacc.Bacc(num_devices=N_CORES)
        with tile.TileContext(nc) as tc:
            build_mha(tc, L=L)
        nc.compile()
        _BUILT[L] = nc
    return _BUILT[L]


def kernel(x, wq, wk, wv, wo):
    from concourse.bass_utils import run_bass_kernel_spmd
    nc = _get_nc()
    in_maps = make_in_maps(x, wq, wk, wv, wo)
    res = run_bass_kernel_spmd(nc, in_maps, core_ids=list(range(N_CORES)))
    y = np.concatenate([res.results[rk]["y"] for rk in range(N_CORES)], axis=0)
    return y.reshape(B, L_FULL, D)



# revision 2
# speedup vs baseline: 51.8646x; 51.8646x over previous
"""Trainium2 Bass kernel for 16-head MHA with RoPE — zero-collective design.

Sharding: each core owns 512 output tokens (batch b = r//4, token offset
(r%4)*512 within the batch) and computes their full attention + o-proj.
The K/V projection for the core's batch (2048 keys, all 16 heads) is
replicated across the 4 cores of that batch — cheaper than any collective
through the PJRT/axon path, and removes all cross-core sync.

All per-core inputs are packed into ONE flat bf16 blob ("blob") because
per-call dispatch cost through the axon PJRT tunnel scales with the
number of argument buffers (~+0.1 ms per extra input per call measured);
on-device views into the blob are free (strided DMA access patterns).

Per-core pipeline (all matmul inputs bf16, PSUM accum f32):
  A) K projection dim-major kT [128, 8, L] and V projection token-major
     v_sb [128, MC, 16, 65] (direct [tok, vdim] matmuls — no transposes),
     Q projection for its 512 tokens. RoPE = x*cos + (R@x)*sin where R is a
     sign-carrying 128x128 block permutation applied on the PE; no
     accumulate-DMAs.
  B) Attention per (head, key-chunk): S^T = kT^T @ qT, exp on ScalarE with
     1/sqrt(hd) folded in (no max subtraction; scores bounded), PV with a
     ones column appended to v so the softmax denominator falls out as
     row 64. PV is issued lagging one chunk so PE never stalls on ScalarE.
  C) o-projection from dim-major aT; y [512, 1024] f32 out.
"""

import numpy as np
from ml_dtypes import bfloat16

# Problem shape (hardcoded per contract - kernel.py must be self-contained)
B, L_FULL, D = 2, 2048, 1024
H, HD = 16, 64
N_CORES = 8
CPB = N_CORES // B            # cores per batch = 4
KC = D // 128                 # contraction chunks = 8
OC = D // 128                 # output partition-chunks = 8


def _rope_tables(L):
    inv_freq = 1.0 / (10000.0 ** (np.arange(0, HD, 2, dtype=np.float64) / HD))
    t = np.arange(L, dtype=np.float64)
    freqs = np.outer(t, inv_freq)                      # [L, 32]
    emb = np.concatenate([freqs, freqs], -1)           # [L, 64]
    cos_t = np.cos(emb).T                              # [64, L]
    sin_t = np.sin(emb).T
    cost = np.concatenate([cos_t, cos_t], 0)           # [128, L] (2 heads)
    sint = np.concatenate([sin_t, sin_t], 0)
    return cost.astype(bfloat16), sint.astype(bfloat16)


def _rot_matrix():
    # rot(x)[o] = -x[o+32] for o in [0,32), +x[o-32] for o in [32,64),
    # per 64-row head block; stationary operand is the transpose R^T[p, o].
    rt = np.zeros((128, 128), dtype=np.float32)
    for blk in range(2):
        base = blk * 64
        for o in range(32):
            rt[base + o + 32, base + o] = -1.0
        for o in range(32, 64):
            rt[base + o - 32, base + o] = 1.0
    return rt.astype(bfloat16)


def _blob_layout(L):
    """(name -> (offset, shape)) for the packed bf16 input blob."""
    T = B * L
    TPC = T // N_CORES
    layout = {}
    off = 0
    for name, shape in (
        ("xt", (D, L)), ("xq", (D, TPC)),
        ("wqt", (D, D)), ("wkt", (D, D)), ("wvt", (D, D)), ("wot", (D, D)),
        ("cosq", (128, TPC)), ("sinq", (128, TPC)),
    ):
        layout[name] = (off, shape)
        off += int(np.prod(shape))
    return layout, off


def build_mha(tc, L=L_FULL, debug=False):
    """Emit the MHA program into TileContext `tc`.

    Per-core DRAM I/O (SPMD-uniform program; all rank differences are data):
      in : blob [NTOT] bf16 — packed xt/xq/wqt/wkt/wvt/wot/cosq/sinq
      out: y [TPC, D] f32
    """
    import concourse.mybir as mybir
    from contextlib import ExitStack

    nc = tc.nc
    f32 = mybir.dt.float32
    bf16 = mybir.dt.bfloat16
    AF = mybir.ActivationFunctionType

    T = B * L
    TPC = T // N_CORES            # query tokens per core
    MC = L // 128                 # key chunks per batch
    CH = min(512, L)              # kv-projection token chunk
    NCH = L // CH                 # chunks over keys
    MPC = CH // 128               # key-chunks per token chunk
    scale = float(HD) ** -0.5

    # ---- I/O ----
    layout, ntot = _blob_layout(L)
    blob_d = nc.dram_tensor("blob", [ntot], bf16, kind="ExternalInput").ap()

    def view(name):
        off, shape = layout[name]
        v = blob_d[off:off + int(np.prod(shape))]
        return v.rearrange("(a b) -> a b", a=shape[0])

    def view_chunked(name):
        # [D, C] source seen as [128, KC, C] (partition-major chunks)
        off, shape = layout[name]
        v = blob_d[off:off + int(np.prod(shape))]
        return v.rearrange("(kk p c) -> p kk c", kk=KC, p=128)

    xt_d = view_chunked("xt")
    xq_d = view_chunked("xq")
    wqt_d = view_chunked("wqt")
    wkt_d = view_chunked("wkt")
    wvt_d = view_chunked("wvt")
    wot_d = view_chunked("wot")
    cosq_d = view("cosq")
    sinq_d = view("sinq")
    y_d = nc.dram_tensor("y", [TPC, D], f32, kind="ExternalOutput").ap()

    # ---- inline constants ----
    cost_np, sint_np = _rope_tables(L)
    cosk_d = nc.inline_tensor(np.ascontiguousarray(cost_np), name="cosk")
    sink_d = nc.inline_tensor(np.ascontiguousarray(sint_np), name="sink")
    rt_d = nc.inline_tensor(_rot_matrix(), name="rotm")

    ctx = ExitStack()
    with ctx:
        # ---------------- persistent pools ----------------
        cpool = ctx.enter_context(tc.tile_pool(name="consts", bufs=1))
        cosk = cpool.tile([128, L], bf16)
        nc.sync.dma_start(cosk[:], cosk_d.ap()[:, :])
        sink = cpool.tile([128, L], bf16)
        nc.sync.dma_start(sink[:], sink_d.ap()[:, :])
        cosq = cpool.tile([128, TPC], bf16)
        nc.sync.dma_start(cosq[:], cosq_d[:, :])
        sinq = cpool.tile([128, TPC], bf16)
        nc.sync.dma_start(sinq[:], sinq_d[:, :])
        rt_sb = cpool.tile([128, 128], bf16)
        nc.sync.dma_start(rt_sb[:], rt_d.ap()[:, :])

        kqpool = ctx.enter_context(tc.tile_pool(name="kq", bufs=1))
        kT = kqpool.tile([128, OC, L], bf16)    # post-RoPE k, dim-major
        qT = kqpool.tile([128, OC, TPC], bf16)  # post-RoPE q, dim-major
        vpool = ctx.enter_context(tc.tile_pool(name="vtm", bufs=1))
        v_sb = vpool.tile([128, MC, H, 65], bf16)  # v token-major + ones col
        nc.gpsimd.memset(v_sb[:, :, :, 64:65], 1.0)

        # ---------------- phase A: projections + RoPE ----------------
        def rope_emit(ps, dst, cos_ap, sin_ap, raw_pool, rot_pool, u_pool, n):
            # dst = ps*cos + (R @ ps)*sin ; raw copy via ScalarE, rot via PE
            raw = raw_pool.tile([128, n], bf16, tag="raw")
            nc.scalar.copy(raw[:], ps[:])
            rot = rot_pool.tile([128, n], f32, tag="rot")
            nc.tensor.matmul(rot[:], rt_sb[:], raw[:], start=True, stop=True)
            nc.vector.tensor_mul(dst, raw[:], cos_ap)
            u = u_pool.tile([128, n], bf16, tag="u")
            nc.vector.tensor_mul(u[:], rot[:], sin_ap)
            nc.vector.tensor_add(dst, dst, u[:])

        with tc.tile_pool(name="wqkv", bufs=1) as wp, \
             tc.tile_pool(name="xs", bufs=2) as xsp, \
             tc.tile_pool(name="raw", bufs=2) as rawp, \
             tc.tile_pool(name="u", bufs=2) as up, \
             tc.tile_pool(name="kps", bufs=2, space="PSUM") as kps, \
             tc.tile_pool(name="rps", bufs=2, space="PSUM") as rps, \
             tc.tile_pool(name="vps", bufs=2, space="PSUM") as vps:
            wq_sb = wp.tile([128, KC, D], bf16)
            wk_sb = wp.tile([128, KC, D], bf16)
            wv_sb = wp.tile([128, KC, D], bf16)
            nc.sync.dma_start(wq_sb[:, :, :], wqt_d[:, :, :])
            nc.sync.dma_start(wk_sb[:, :, :], wkt_d[:, :, :])
            nc.sync.dma_start(wv_sb[:, :, :], wvt_d[:, :, :])

            # Q projection (dim-major) for this core's TPC tokens
            xq_sb = xsp.tile([128, KC, TPC], bf16, tag="xq", name="xq_sb")
            nc.sync.dma_start(xq_sb[:, :, :], xq_d[:, :, :])
            for oc in range(OC):
                q_ps = kps.tile([128, TPC], f32, tag="qk_ps")
                for kk in range(KC):
                    nc.tensor.matmul(q_ps[:],
                                     wq_sb[:, kk, oc * 128:(oc + 1) * 128],
                                     xq_sb[:, kk, :],
                                     start=(kk == 0), stop=(kk == KC - 1))
                rope_emit(q_ps, qT[:, oc, :], cosq[:], sinq[:],
                          rawp, rps, up, TPC)

            # K (dim-major + RoPE) and V (token-major) per token chunk
            for tcn in range(NCH):
                cols = slice(tcn * CH, (tcn + 1) * CH)
                xt_sb = xsp.tile([128, KC, CH], bf16, tag="xt")
                nc.sync.dma_start(xt_sb[:, :, :], xt_d[:, :, cols])
                for oc in range(OC):
                    k_ps = kps.tile([128, CH], f32, tag="qk_ps")
                    for kk in range(KC):
                        nc.tensor.matmul(k_ps[:],
                                         wk_sb[:, kk, oc * 128:(oc + 1) * 128],
                                         xt_sb[:, kk, :],
                                         start=(kk == 0), stop=(kk == KC - 1))
                    rope_emit(k_ps, kT[:, oc, cols], cosk[:, cols],
                              sink[:, cols], rawp, rps, up, CH)
                for mi in range(MPC):
                    m = tcn * MPC + mi
                    ts = slice(mi * 128, (mi + 1) * 128)
                    v_ps = vps.tile([128, H, 64], f32, tag="v_ps")
                    for kk in range(KC):
                        for hf in range(2):
                            nc.tensor.matmul(
                                v_ps[:, hf * 8:(hf + 1) * 8, :],
                                xt_sb[:, kk, ts],
                                wv_sb[:, kk, hf * 512:(hf + 1) * 512],
                                start=(kk == 0), stop=(kk == KC - 1))
                    nc.vector.tensor_copy(v_sb[:, m, :, 0:64], v_ps[:])

        # ---------------- phase B: attention ----------------
        apool = ctx.enter_context(tc.tile_pool(name="aT", bufs=1))
        aT = apool.tile([128, OC, TPC], bf16)

        wop = ctx.enter_context(tc.tile_pool(name="wo", bufs=1))
        wo_sb = wop.tile([128, KC, D], bf16)
        nc.sync.dma_start(wo_sb[:, :, :], wot_d[:, :, :])

        with tc.tile_pool(name="pt", bufs=3) as ptp, \
             tc.tile_pool(name="ep", bufs=2) as epool, \
             tc.tile_pool(name="stp", bufs=3, space="PSUM") as stp, \
             tc.tile_pool(name="oup", bufs=2, space="PSUM") as oup:
            for h in range(H):
                po, pc = (h % 2) * 64, h // 2
                hs = slice(po, po + 64)
                outU = oup.tile([65, TPC], f32, tag="outU")
                pend = None  # lag-1 PV issue keeps PE off the ScalarE chain
                for m in range(MC):
                    ks = slice(m * 128, (m + 1) * 128)
                    st = stp.tile([128, TPC], f32, tag="st")
                    nc.tensor.matmul(st[:], kT[hs, pc, ks], qT[hs, pc, :],
                                     start=True, stop=True)
                    pt = ptp.tile([128, TPC], bf16, tag="pt")
                    nc.scalar.activation(pt[:], st[:], AF.Exp, scale=scale)
                    if pend is not None:
                        pm, ppt = pend
                        nc.tensor.matmul(outU[:], v_sb[:, pm, h, :], ppt[:],
                                         start=(pm == 0), stop=False)
                    pend = (m, pt)
                pm, ppt = pend
                nc.tensor.matmul(outU[:], v_sb[:, pm, h, :], ppt[:],
                                 start=(pm == 0), stop=True)
                # normalize: denominator is row 64 of outU
                dinv = epool.tile([1, TPC], f32, tag="dinv")
                nc.vector.reciprocal(dinv[:], outU[64:65, :])
                bc = epool.tile([64, TPC], f32, tag="bc")
                nc.gpsimd.partition_broadcast(bc[:], dinv[:])
                nc.vector.tensor_mul(aT[hs, pc, :], outU[0:64, :], bc[:])

        # ---------------- phase C: o-projection ----------------
        MT = min(128, TPC)
        with tc.tile_pool(name="ysb", bufs=2) as ysp, \
             tc.tile_pool(name="yps", bufs=2, space="PSUM") as yps:
            for mt in range(TPC // MT):
                ms = slice(mt * MT, (mt + 1) * MT)
                y_ps = yps.tile([MT, D], f32, tag="y_ps")
                for kk in range(KC):
                    for no in range(2):
                        nc.tensor.matmul(
                            y_ps[:, no * 512:(no + 1) * 512],
                            aT[:, kk, ms],
                            wo_sb[:, kk, no * 512:(no + 1) * 512],
                            start=(kk == 0), stop=(kk == KC - 1))
                y_sb = ysp.tile([MT, D], f32, tag="y_sb")
                nc.vector.tensor_copy(y_sb[:], y_ps[:])
                nc.sync.dma_start(y_d[ms, :], y_sb[:])

    return nc


def make_in_maps(x, wq, wk, wv, wo, L=L_FULL):
    T = B * L
    TPC = T // N_CORES
    x3 = np.asarray(x, dtype=np.float32).reshape(B, L, D)
    xt_b = [np.ascontiguousarray(x3[b].T).astype(bfloat16) for b in range(B)]
    wqt = np.ascontiguousarray(np.asarray(wq, np.float32).T).astype(bfloat16)
    wkt = np.ascontiguousarray(np.asarray(wk, np.float32).T).astype(bfloat16)
    wvt = np.ascontiguousarray(np.asarray(wv, np.float32).T).astype(bfloat16)
    wot = np.ascontiguousarray(np.asarray(wo, np.float32).T).astype(bfloat16)
    cost_np, sint_np = _rope_tables(L)
    layout, ntot = _blob_layout(L)
    in_maps = []
    for r in range(N_CORES):
        b = r // CPB
        qoff = (r % CPB) * TPC
        qs = slice(qoff, qoff + TPC)
        parts = {
            "xt": xt_b[b],
            "xq": xt_b[b][:, qs],
            "wqt": wqt, "wkt": wkt, "wvt": wvt, "wot": wot,
            "cosq": cost_np[:, qs],
            "sinq": sint_np[:, qs],
        }
        blob = np.empty(ntot, dtype=bfloat16)
        for name, (off, shape) in layout.items():
            blob[off:off + int(np.prod(shape))] = np.ascontiguousarray(
                parts[name]).ravel()
        in_maps.append({"blob": blob})
    return in_maps


_BUILT = {}


def _get_nc(L=L_FULL):
    if L not in _BUILT:
        import concourse.tile as tile
        from concourse import bacc
        nc = bacc.Bacc(num_devices=N_CORES)
        with tile.TileContext(nc) as tc:
            build_mha(tc, L=L)
        nc.compile()
        _BUILT[L] = nc
    return _BUILT[L]


def kernel(x, wq, wk, wv, wo):
    from concourse.bass_utils import run_bass_kernel_spmd
    nc = _get_nc()
    in_maps = make_in_maps(x, wq, wk, wv, wo)
    res = run_bass_kernel_spmd(nc, in_maps, core_ids=list(range(N_CORES)))
    TPC = B * L_FULL // N_CORES
    y = np.empty((B, L_FULL, D), np.float32)
    for r in range(N_CORES):
        b = r // CPB
        qoff = (r % CPB) * TPC
        y[b, qoff:qoff + TPC] = res.results[r]["y"]
    return y


# revision 3
# speedup vs baseline: 72.0235x; 1.3887x over previous
"""Trainium2 Bass kernel for 16-head MHA with RoPE — zero-collective,
software-pipelined design.

Sharding: each core owns 512 output tokens (batch b = r//4, token offset
(r%4)*512 within the batch) and computes their full attention + o-proj.
The K/V projection for the core's batch (2048 keys, all 16 heads) is
replicated across the 4 cores of that batch — cheaper than any collective
through the PJRT/axon path, and removes all cross-core sync.

All per-core inputs are packed into ONE flat bf16 blob ("blob") because
per-call dispatch cost through the axon PJRT tunnel scales with the
number of argument buffers (~+0.1 ms per extra input per call measured).

The emission is software-pipelined across engines: attention for key-group
g (ScalarE-heavy: exp) is interleaved at head granularity with the K/V
projection matmuls of key-group g+1 (PE-heavy), so neither engine idles;
the o-projection is interleaved into the last group's attention as each
head-pair's output becomes available. Attention output accumulates across
key-groups in SBUF (row 64 = softmax denominator via a ones column in v).
RoPE = x*cos + (R@x)*sin with R a sign-carrying 128x128 block permutation
applied on the PE.
"""

import numpy as np
from ml_dtypes import bfloat16

# Problem shape (hardcoded per contract - kernel.py must be self-contained)
B, L_FULL, D = 2, 2048, 1024
H, HD = 16, 64
N_CORES = 8
CPB = N_CORES // B            # cores per batch = 4
KC = D // 128                 # contraction chunks = 8
OC = D // 128                 # output partition-chunks = 8


def _rope_tables(L):
    inv_freq = 1.0 / (10000.0 ** (np.arange(0, HD, 2, dtype=np.float64) / HD))
    t = np.arange(L, dtype=np.float64)
    freqs = np.outer(t, inv_freq)                      # [L, 32]
    emb = np.concatenate([freqs, freqs], -1)           # [L, 64]
    cos_t = np.cos(emb).T                              # [64, L]
    sin_t = np.sin(emb).T
    cost = np.concatenate([cos_t, cos_t], 0)           # [128, L] (2 heads)
    sint = np.concatenate([sin_t, sin_t], 0)
    return cost.astype(bfloat16), sint.astype(bfloat16)


def _rot_matrix():
    # rot(x)[o] = -x[o+32] for o in [0,32), +x[o-32] for o in [32,64),
    # per 64-row head block; stationary operand is the transpose R^T[p, o].
    rt = np.zeros((128, 128), dtype=np.float32)
    for blk in range(2):
        base = blk * 64
        for o in range(32):
            rt[base + o + 32, base + o] = -1.0
        for o in range(32, 64):
            rt[base + o - 32, base + o] = 1.0
    return rt.astype(bfloat16)


def _blob_layout(L):
    """(name -> (offset, shape)) for the packed bf16 input blob.

    Each core's key axis is rotated by its query offset (host-side
    np.roll of xt and the RoPE tables together) — softmax is invariant
    to key order, and the rotation makes the core's query tokens always
    columns [0, TPC) of xt, so no separate xq/cosq/sinq regions exist.
    """
    layout = {}
    off = 0
    for name, shape in (
        ("xt", (D, L)),
        ("wqt", (D, D)), ("wkt", (D, D)), ("wvt", (D, D)), ("wot", (D, D)),
        ("cosk", (128, L)), ("sink", (128, L)),
    ):
        layout[name] = (off, shape)
        off += int(np.prod(shape))
    return layout, off


def build_mha(tc, L=L_FULL, debug=False):
    """Emit the MHA program into TileContext `tc`.

    Per-core DRAM I/O (SPMD-uniform program; all rank differences are data):
      in : blob [NTOT] bf16 — packed xt/xq/wqt/wkt/wvt/wot/cosq/sinq
      out: y [TPC, D] f32
    """
    import concourse.mybir as mybir
    from contextlib import ExitStack

    nc = tc.nc
    f32 = mybir.dt.float32
    bf16 = mybir.dt.bfloat16
    AF = mybir.ActivationFunctionType

    T = B * L
    TPC = T // N_CORES            # query tokens per core
    MC = L // 128                 # key chunks per batch
    CH = min(512, L)              # kv-projection token chunk (key group)
    NCH = L // CH                 # key groups
    MPC = CH // 128               # key-chunks per group
    MT = min(128, TPC)            # o-proj token tile
    NMT = TPC // MT
    scale = float(HD) ** -0.5
    assert CH >= TPC or L <= 512  # queries live in group 0's columns
    # Sections redistribute the attention chunks so each section's ScalarE
    # (exp) work fits under its PE envelope: early sections carry extra kv
    # projection, the last section carries the o-projection.
    if NCH == 4:
        SEC_CHUNKS = [list(range(0, 4)), list(range(4, 12)),
                      list(range(12, 16))]
        SEC_KV = [[1, 2], [3], []]
    else:
        SEC_CHUNKS = [list(range(MC))]
        SEC_KV = [[]]

    # ---- I/O ----
    layout, ntot = _blob_layout(L)
    blob_d = nc.dram_tensor("blob", [ntot], bf16, kind="ExternalInput").ap()

    def view(name):
        off, shape = layout[name]
        v = blob_d[off:off + int(np.prod(shape))]
        return v.rearrange("(a b) -> a b", a=shape[0])

    def view_chunked(name):
        # [D, C] source seen as [128, KC, C] (partition-major chunks)
        off, shape = layout[name]
        v = blob_d[off:off + int(np.prod(shape))]
        return v.rearrange("(kk p c) -> p kk c", kk=KC, p=128)

    xt_d = view_chunked("xt")
    wqt_d = view_chunked("wqt")
    wkt_d = view_chunked("wkt")
    wvt_d = view_chunked("wvt")
    wot_d = view_chunked("wot")
    cosk_d = view("cosk")
    sink_d = view("sink")
    y_d = nc.dram_tensor("y", [TPC, D], f32, kind="ExternalOutput").ap()

    # ---- inline constants ----
    rt_d = nc.inline_tensor(_rot_matrix(), name="rotm")

    ctx = ExitStack()
    with ctx:
        # ---------------- persistent pools ----------------
        # (input DMAs are issued in consumption order: xt chunk 0 + wq feed
        # the q-projection, then cos/sin, then wk/wv/wo for the kv pipeline)
        cpool = ctx.enter_context(tc.tile_pool(name="consts", bufs=1))
        cosk = cpool.tile([128, L], bf16)
        sink = cpool.tile([128, L], bf16)
        rt_sb = cpool.tile([128, 128], bf16)

        kqpool = ctx.enter_context(tc.tile_pool(name="kq", bufs=1))
        kT = kqpool.tile([128, OC, L], bf16)    # post-RoPE k, dim-major
        qT = kqpool.tile([128, OC, TPC], bf16)  # post-RoPE q, dim-major
        vpool = ctx.enter_context(tc.tile_pool(name="vtm", bufs=1))
        v_sb = vpool.tile([128, MC, H, 65], bf16)  # v token-major + ones col
        nc.gpsimd.memset(v_sb[:, :, :, 64:65], 1.0)
        aupool = ctx.enter_context(tc.tile_pool(name="aU", bufs=1))
        aU = aupool.tile([65, H, TPC], bf16)    # attention accum across groups
        apool = ctx.enter_context(tc.tile_pool(name="aT", bufs=1))
        aT = apool.tile([128, OC, TPC], bf16)   # normalized, dim-major
        ypool = ctx.enter_context(tc.tile_pool(name="yacc", bufs=1))
        y_acc = ypool.tile([MT, NMT, D], f32)   # o-proj accum across kk

        wpool = ctx.enter_context(tc.tile_pool(name="w", bufs=1))
        wk_sb = wpool.tile([128, KC, D], bf16)
        wv_sb = wpool.tile([128, KC, D], bf16)
        wo_sb = wpool.tile([128, KC, D], bf16)

        # x stream pool (persistent; holds the current key-group chunk)
        xsp = ctx.enter_context(tc.tile_pool(name="xs", bufs=2))

        def rope_emit(ps, dst, cos_ap, sin_ap, n, rawp, up, rps):
            # dst = ps*cos + (R @ ps)*sin ; raw copy via ScalarE, rot via PE
            raw = rawp.tile([128, n], bf16, tag="raw")
            nc.scalar.copy(raw[:], ps[:])
            rot = rps.tile([128, n], f32, tag="rot")
            nc.tensor.matmul(rot[:], rt_sb[:], raw[:], start=True, stop=True)
            nc.vector.tensor_mul(dst, raw[:], cos_ap)
            u = up.tile([128, n], bf16, tag="u")
            nc.vector.tensor_mul(u[:], rot[:], sin_ap)
            nc.vector.tensor_add(dst, dst, u[:])

        # ------- q-projection (queries are xt cols [0, TPC); wq freed) ------
        xt0_sb = xsp.tile([128, KC, CH], bf16, tag="xt")
        nc.sync.dma_start(xt0_sb[:, :, :], xt_d[:, :, 0:CH])
        with tc.tile_pool(name="wqp", bufs=1) as wqp, \
             tc.tile_pool(name="qraw", bufs=2) as qrawp, \
             tc.tile_pool(name="qu", bufs=2) as qup, \
             tc.tile_pool(name="qps", bufs=2, space="PSUM") as qps, \
             tc.tile_pool(name="qrps", bufs=1, space="PSUM") as qrps:
            wq_sb = wqp.tile([128, KC, D], bf16)
            nc.sync.dma_start(wq_sb[:, :, :], wqt_d[:, :, :])
            nc.sync.dma_start(cosk[:], cosk_d[:, :])
            nc.sync.dma_start(sink[:], sink_d[:, :])
            nc.sync.dma_start(rt_sb[:], rt_d.ap()[:, :])
            nc.sync.dma_start(wk_sb[:, :, :], wkt_d[:, :, :])
            nc.sync.dma_start(wv_sb[:, :, :], wvt_d[:, :, :])
            nc.sync.dma_start(wo_sb[:, :, :], wot_d[:, :, :])
            for oc in range(OC):
                q_ps = qps.tile([128, TPC], f32, tag="q_ps")
                for kk in range(KC):
                    nc.tensor.matmul(q_ps[:],
                                     wq_sb[:, kk, oc * 128:(oc + 1) * 128],
                                     xt0_sb[:, kk, 0:TPC],
                                     start=(kk == 0), stop=(kk == KC - 1))
                rope_emit(q_ps, qT[:, oc, :], cosk[:, 0:TPC], sink[:, 0:TPC],
                          TPC, qrawp, qup, qrps)

        # attention pools (outlive the kv pools; closed after the last group)
        attn_stack = ExitStack()
        ptp = attn_stack.enter_context(tc.tile_pool(name="pt", bufs=3))
        epool = attn_stack.enter_context(tc.tile_pool(name="ep", bufs=2))
        stp = attn_stack.enter_context(tc.tile_pool(name="stp", bufs=2, space="PSUM"))
        oup = attn_stack.enter_context(tc.tile_pool(name="oup", bufs=1, space="PSUM"))

        # kv-projection pools (closed once the last group is projected)
        kvstack = ExitStack()
        rawp = kvstack.enter_context(tc.tile_pool(name="raw", bufs=2))
        up = kvstack.enter_context(tc.tile_pool(name="u", bufs=2))
        kps = kvstack.enter_context(tc.tile_pool(name="kps", bufs=2, space="PSUM"))
        rps = kvstack.enter_context(tc.tile_pool(name="rps", bufs=1, space="PSUM"))
        vps = kvstack.enter_context(tc.tile_pool(name="vps", bufs=1, space="PSUM"))

        def emit_kv_pieces(g, xt_pre=None):
            """Generator of closures; each emits one PE-sized piece of the
            K/V projection + RoPE for key group g."""
            cols = slice(g * CH, (g + 1) * CH)
            if xt_pre is None:
                xt_sb = xsp.tile([128, KC, CH], bf16, tag="xt")

                def load():
                    nc.sync.dma_start(xt_sb[:, :, :], xt_d[:, :, cols])
                yield load
            else:
                xt_sb = xt_pre
            for oc in range(OC):
                def kproj(oc=oc):
                    k_ps = kps.tile([128, CH], f32, tag="qk_ps")
                    for kk in range(KC):
                        nc.tensor.matmul(k_ps[:],
                                         wk_sb[:, kk, oc * 128:(oc + 1) * 128],
                                         xt_sb[:, kk, :],
                                         start=(kk == 0), stop=(kk == KC - 1))
                    rope_emit(k_ps, kT[:, oc, cols], cosk[:, cols],
                              sink[:, cols], CH, rawp, up, rps)
                yield kproj
            for mi in range(MPC):
                def vproj(mi=mi):
                    m = g * MPC + mi
                    ts = slice(mi * 128, (mi + 1) * 128)
                    v_ps = vps.tile([128, H, 64], f32, tag="v_ps")
                    for kk in range(KC):
                        for hf in range(2):
                            nc.tensor.matmul(
                                v_ps[:, hf * 8:(hf + 1) * 8, :],
                                xt_sb[:, kk, ts],
                                wv_sb[:, kk, hf * 512:(hf + 1) * 512],
                                start=(kk == 0), stop=(kk == KC - 1))
                    nc.vector.tensor_copy(v_sb[:, m, :, 0:64], v_ps[:])
                yield vproj

        # ---------------- pipelined attention + kv + o-proj ----------------
        def attn_head(chunks, first, h):
            """Scores + exp + PV for head h over the listed key chunks;
            accumulate into aU[:, h, :]."""
            po, pc = (h % 2) * 64, h // 2
            hs = slice(po, po + 64)
            outU = oup.tile([65, TPC], f32, tag="outU")
            pend = None
            for j, m in enumerate(chunks):
                ks = slice(m * 128, (m + 1) * 128)
                st = stp.tile([128, TPC], f32, tag="st")
                nc.tensor.matmul(st[:], kT[hs, pc, ks], qT[hs, pc, :],
                                 start=True, stop=True)
                pt = ptp.tile([128, TPC], bf16, tag="pt")
                nc.scalar.activation(pt[:], st[:], AF.Exp, scale=scale)
                if pend is not None:
                    pj, pm, ppt = pend
                    nc.tensor.matmul(outU[:], v_sb[:, pm, h, :], ppt[:],
                                     start=(pj == 0), stop=False)
                pend = (j, m, pt)
            pj, pm, ppt = pend
            nc.tensor.matmul(outU[:], v_sb[:, pm, h, :], ppt[:],
                             start=(pj == 0), stop=True)
            if first:
                nc.vector.tensor_copy(aU[:, h, :], outU[:])
            else:
                # bf16 accumulation over only 3 partial sums; error is
                # ~0.4% against a 2e-2 correctness budget
                with nc.allow_low_precision(reason="3-way bf16 attn accum"):
                    nc.vector.tensor_add(aU[:, h, :], aU[:, h, :], outU[:])

        def normalize_quad(q):
            # batched reciprocal of 4 heads' denominators into a partition-0
            # tile (partition_broadcast sources partition 0 on hardware),
            # then broadcast + scale each head into dim-major aT (all bf16)
            hq = slice(4 * q, 4 * q + 4)
            dinv = epool.tile([1, 4, TPC], bf16, tag="dinv")
            with nc.allow_low_precision(reason="bf16 softmax denom recip"):
                nc.vector.reciprocal(dinv[:], aU[64:65, hq, :])
            for j, h in enumerate(range(4 * q, 4 * q + 4)):
                po, pc = (h % 2) * 64, h // 2
                hs = slice(po, po + 64)
                bc = epool.tile([64, TPC], bf16, tag="bc")
                nc.gpsimd.partition_broadcast(bc[:], dinv[0:1, j, :])
                nc.vector.tensor_mul(aT[hs, pc, :], aU[0:64, h, :], bc[:])

        # group 0 kv-projection runs un-overlapped (nothing to hide it behind)
        for piece in emit_kv_pieces(0, xt_pre=xt0_sb):
            piece()

        yps_stack = ExitStack()

        def oproj_piece(half, yps):
            """o-projection over a kk-quad (heads 8*half..8*half+7), PSUM
            accumulated, then one add per token tile into y_acc."""
            kks = range(4 * half, 4 * half + 4)
            for mt in range(NMT):
                ms = slice(mt * MT, (mt + 1) * MT)
                y_ps = yps.tile([MT, D], f32, tag="y_ps")
                for j, kk in enumerate(kks):
                    for no in range(2):
                        nc.tensor.matmul(y_ps[:, no * 512:(no + 1) * 512],
                                         aT[:, kk, ms],
                                         wo_sb[:, kk, no * 512:(no + 1) * 512],
                                         start=(j == 0), stop=(j == 3))
                if half == 0:
                    nc.vector.tensor_copy(y_acc[:, mt, :], y_ps[:])
                else:
                    nc.vector.tensor_add(y_acc[:, mt, :], y_acc[:, mt, :],
                                         y_ps[:])

        for s, chunks in enumerate(SEC_CHUNKS):
            last = (s == len(SEC_CHUNKS) - 1)
            pieces = []
            for g in SEC_KV[s]:
                pieces.extend(emit_kv_pieces(g))
            if last:
                # all projections emitted; free kv PSUM for o-proj tiles
                kvstack.close()
                yps = yps_stack.enter_context(
                    tc.tile_pool(name="yps", bufs=2, space="PSUM"))
            pi = 0
            npc = len(pieces)
            for h in range(H):
                attn_head(chunks, s == 0, h)
                # spread the kv pieces of later groups evenly over the heads
                want = (h + 1) * npc // H
                while pi < want:
                    pieces[pi]()
                    pi += 1
                if last:
                    if h % 4 == 3:
                        normalize_quad(h // 4)
                    if h % 8 == 7:
                        oproj_piece(h // 8, yps)

        yps_stack.close()
        attn_stack.close()

        # ---------------- y writeback ----------------
        for mt in range(NMT):
            nc.sync.dma_start(y_d[mt * MT:(mt + 1) * MT, :], y_acc[:, mt, :])

    return nc


def make_in_maps(x, wq, wk, wv, wo, L=L_FULL):
    T = B * L
    TPC = T // N_CORES
    x3 = np.asarray(x, dtype=np.float32).reshape(B, L, D)
    xt_b = [np.ascontiguousarray(x3[b].T).astype(bfloat16) for b in range(B)]
    wqt = np.ascontiguousarray(np.asarray(wq, np.float32).T).astype(bfloat16)
    wkt = np.ascontiguousarray(np.asarray(wk, np.float32).T).astype(bfloat16)
    wvt = np.ascontiguousarray(np.asarray(wv, np.float32).T).astype(bfloat16)
    wot = np.ascontiguousarray(np.asarray(wo, np.float32).T).astype(bfloat16)
    cost_np, sint_np = _rope_tables(L)
    layout, ntot = _blob_layout(L)
    in_maps = []
    for r in range(N_CORES):
        b = r // CPB
        qoff = (r % CPB) * TPC
        # rotate the key axis so this core's queries sit at columns [0, TPC);
        # softmax is key-order invariant, so rotating x and the RoPE tables
        # together is exact.
        parts = {
            "xt": np.roll(xt_b[b], -qoff, axis=1),
            "wqt": wqt, "wkt": wkt, "wvt": wvt, "wot": wot,
            "cosk": np.roll(cost_np, -qoff, axis=1),
            "sink": np.roll(sint_np, -qoff, axis=1),
        }
        blob = np.empty(ntot, dtype=bfloat16)
        for name, (off, shape) in layout.items():
            blob[off:off + int(np.prod(shape))] = np.ascontiguousarray(
                parts[name]).ravel()
        in_maps.append({"blob": blob})
    return in_maps


_BUILT = {}


def _get_nc(L=L_FULL):
    if L not in _BUILT:
        import concourse.tile as tile
        from concourse import bacc
        nc = bacc.Bacc(num_devices=N_CORES)
        with tile.TileContext(nc) as tc:
            build_mha(tc, L=L)
        nc.compile()
        _BUILT[L] = nc
    return _BUILT[L]


def kernel(x, wq, wk, wv, wo):
    from concourse.bass_utils import run_bass_kernel_spmd
    nc = _get_nc()
    in_maps = make_in_maps(x, wq, wk, wv, wo)
    res = run_bass_kernel_spmd(nc, in_maps, core_ids=list(range(N_CORES)))
    TPC = B * L_FULL // N_CORES
    y = np.empty((B, L_FULL, D), np.float32)
    for r in range(N_CORES):
        b = r // CPB
        qoff = (r % CPB) * TPC
        y[b, qoff:qoff + TPC] = res.results[r]["y"]
    return y
